# revision 7
# baseline (speedup 1.0000x reference)
"""DLRM dot-interaction kernel for Trainium2 (8 NeuronCores, batch-sharded).

Computes, per sample b: T = concat(dense[b], embs[b]) -> [27, 128];
Z = T @ T^T; output = strict upper triangle of Z -> [351] floats.

Strategy per core (2048 samples):
  - Load blocks of 128 samples as [128 part=b, 27*128 free=(f,d)] fp16
    (SWDGE DMA casts fp32->fp16 on the fly).
  - PE-transpose each [128b, 128d] feature slab -> PSUM [128d, 128b],
    copy into SBUF Tt [128 d, (f,b)] fp16 (DVE/ACT split).
  - Per-sample matmul (fp16): lhsT = rhs = Tt[:, b, :] ([128 d, 27 f]
    strided AP); out -> PSUM [27, 27] at partition offset 32*(b%4)
    (col-group tiling packs 4 samples per PSUM partition dim).
  - Copy PSUM Z -> SBUF Zs [128 part=(g,m), (q,n)] fp32 (ACT).
  - Strict-upper-triangle extraction done by 26 strided DMAs per
    512-sample super-block, one per row m: row m contributes the
    contiguous run z[m, m+1:27] -> out[b, off_m : off_m+26-m].
"""

import numpy as np

B, NUM_EMBS, D = 16384, 26, 128
N_CORES = 8
BC = B // N_CORES  # 2048 samples per core
BLK = 128          # samples per block
NF = NUM_EMBS + 1  # 27 features
NPAIR = NF * (NF - 1) // 2  # 351

_CACHE = {}


def build(bc=BC):
    import concourse.bacc as bacc
    import concourse.mybir as mybir
    from concourse.tile import TileContext
    from concourse.masks import make_identity

    fp16 = mybir.dt.float16
    fp32 = mybir.dt.float32

    nc = bacc.Bacc("TRN2", target_bir_lowering=False, debug=False)
    dense_t = nc.dram_tensor("dense", (bc, D), fp32, kind="ExternalInput")
    embs_t = nc.dram_tensor("embs", (bc, NUM_EMBS, D), fp32, kind="ExternalInput")
    out_t = nc.dram_tensor("out", (bc, NPAIR), fp32, kind="ExternalOutput")

    nblk = bc // BLK
    SBLK = 4 if nblk % 4 == 0 else 1  # blocks per output super-block
    QG = 16                           # 4-sample groups per PSUM Z tile

    with TileContext(nc) as tc:
        with (
            tc.tile_pool(name="consts", bufs=1) as consts,
            tc.tile_pool(name="xin", bufs=2) as xpool,
            tc.tile_pool(name="tt", bufs=2) as ttpool,
            tc.tile_pool(name="zsb", bufs=2) as zpool,
            tc.tile_pool(name="tp", bufs=3, space="PSUM") as tppool,
            tc.tile_pool(name="zp", bufs=3, space="PSUM") as zppool,
            tc.tile_pool(name="dscr", bufs=2, space="DRAM") as dpool,
        ):
            ident = consts.tile([128, 128], fp16)
            make_identity(nc, ident)

            dview = dense_t.ap()  # [bc, 128]
            eview = embs_t.ap().rearrange("b f d -> b (f d)")  # [bc, 3328]
            oview = out_t.ap()  # [bc, 351]

            FP = 32  # per-sample feature pitch in Tt / Z columns (27 used + 5 pad)
            for sblk in range(nblk // SBLK):
                # Z results for the whole super-block, [(g,m) part, (q,n)]
                Zs = zpool.tile([128, SBLK * (BLK // 4) * FP], fp32, tag="Zs")
                for blki in range(SBLK):
                    blk = sblk * SBLK + blki
                    b0 = blk * BLK
                    # ---- load block, cast fp32 -> fp16 ----
                    X = xpool.tile([BLK, NF * D], fp16, tag="X")
                    nc.gpsimd.dma_start(out=X[:, 0:D], in_=dview[b0 : b0 + BLK])
                    nc.gpsimd.dma_start(out=X[:, D:], in_=eview[b0 : b0 + BLK])

                    # ---- transpose to Tt [128 d, (b, f)] with pitch-32 f ----
                    Tt = ttpool.tile([128, BLK * FP], fp16, tag="Tt")
                    Tt3 = Tt.rearrange("d (b f) -> d f b", f=FP)  # [128, 32, 128]
                    # zero the 5 pad feature slots per sample
                    nc.gpsimd.memset(Tt3[:, NF:FP, :], 0.0)
                    nchunk = (NF + 3) // 4
                    for ci in range(nchunk):
                        c0 = ci * 4
                        cf = min(4, NF - c0)
                        tp = tppool.tile([128, 4 * BLK], fp16, tag="tp")
                        for j in range(cf):
                            f = c0 + j
                            nc.tensor.transpose(
                                tp[:, j * BLK : (j + 1) * BLK],
                                X[:, f * D : (f + 1) * D],
                                ident,
                            )
                        # copy PSUM -> SBUF; split chunks between DVE and ACT
                        src = tp[:, : cf * BLK].rearrange("d (f b) -> d f b", b=BLK)
                        dst = Tt3[:, c0 : c0 + cf, :]
                        if ci % 7 < 4:
                            nc.vector.tensor_copy(out=dst, in_=src)
                        else:
                            nc.scalar.copy(dst, src)

                    # ---- per-sample Gram matmuls ----
                    nq = BLK // 4  # 32 groups of 4 samples
                    for qt in range(0, nq, QG):
                        qg = min(QG, nq - qt)
                        zp = zppool.tile([128, QG * FP], fp32, tag="zp")
                        for q in range(qg):
                            for g in range(4):
                                bloc = (qt + q) * 4 + g
                                op = Tt[:, bloc * FP : (bloc + 1) * FP]  # [128, 32]
                                nc.tensor.matmul(
                                    zp[32 * g : 32 * (g + 1), q * FP : (q + 1) * FP],
                                    op,
                                    op,
                                    start=True,
                                    stop=True,
                                    tile_position=(0, 32 * g),
                                )
                        # copy Z PSUM -> SBUF super-block buffer (ACT)
                        zcol0 = (blki * nq + qt) * FP
                        nc.scalar.copy(
                            Zs[:, zcol0 : zcol0 + qg * FP],
                            zp[:, : qg * FP],
                        )

                # ---- strict-upper-triangle extraction ----
                # Bounce Z through DRAM scratch (SBUF partition addressing
                # does not support per-m partition bases), then gather the
                # triu rows with pure-DRAM strided DMAs.
                sb0 = sblk * SBLK * BLK
                nsamp = SBLK * BLK
                nqs = nsamp // 4
                scr = dpool.tile([128, nqs * FP], fp32, tag="scr")
                nc.sync.dma_start(out=scr[:, :], in_=Zs[:, :])
                scr4 = scr.rearrange(
                    "(g m) (q n) -> g m q n", g=4, n=FP
                )  # [4, 32, nqs, 32]
                ov = oview[sb0 : sb0 + nsamp]  # [nsamp, 351]
                ovr = ov.rearrange("(q g) p -> g q p", g=4)  # [4, nqs, 351]
                off = 0
                for m in range(NF - 1):
                    ln = NF - 1 - m
                    src = scr4[:, m, :, m + 1 : NF]  # [4, nqs, ln]
                    dst = ovr[:, :, off : off + ln]
                    nc.sync.dma_start(out=dst, in_=src)
                    off += ln

    nc.compile()
    return nc


def _get(bc=BC):
    if bc not in _CACHE:
        _CACHE[bc] = build(bc)
    return _CACHE[bc]


def kernel(dense: np.ndarray, embs: np.ndarray) -> np.ndarray:
    from concourse import bass_utils

    dense = np.ascontiguousarray(np.asarray(dense, dtype=np.float32))
    embs = np.ascontiguousarray(np.asarray(embs, dtype=np.float32))
    assert dense.shape == (B, D) and embs.shape == (B, NUM_EMBS, D)

    nc = _get()
    dsh = dense.reshape(N_CORES, BC, D)
    esh = embs.reshape(N_CORES, BC, NUM_EMBS, D)
    in_maps = [{"dense": dsh[i], "embs": esh[i]} for i in range(N_CORES)]
    res = bass_utils.run_bass_kernel_spmd(nc, in_maps, core_ids=list(range(N_CORES)))
    return np.concatenate([r["out"] for r in res.results], axis=0)


# revision 30
# speedup vs baseline: 1.1469x; 1.1469x over previous
"""DLRM dot-interaction kernel for Trainium2 (8 NeuronCores, batch-sharded).

Per sample b: T = concat(dense[b], embs[b]) -> [27, 128]; Z = T @ T^T;
output = strict upper triangle of Z -> [351] fp32.

Per-core plan (2048 samples, 16 blocks of 128):
  - SWDGE cast-DMA loads 2 blocks at a time as [128 b, (f,d)] fp16.
  - PE transposes each [128 b, 128 d] feature slab into PSUM; DVE/ACT copy
    into f-major Tt [128 d, f*128+b] fp16 (contiguous copies).
  - Per-sample fp16 matmul: lhsT = rhs = strided AP [128 d, 32 f] (27 real
    features + 5 zero pads); out -> PSUM [32, 32] at partition 32*(b%4)
    (col-group tiling, 4 samples per PSUM partition dim).
  - ACT copies Z PSUM -> SBUF Zs [(g,m) part, (blk,q,n)] fp32, half-core span.
  - Triu extraction: SWDGE bounces Zs to DRAM scratch (full rows, big
    descriptors); then per (m, half) one HWDGE DRAM->DRAM gather DMA with
    1024 descriptors (spreads over ~8-16 DMA engines) packs z[m, m+1:27]
    runs into out[b, off_m:...].
"""

import numpy as np

B, NUM_EMBS, D = 16384, 26, 128
N_CORES = 8
BC = B // N_CORES  # 2048 samples per core
BLK = 128          # samples per block
NF = NUM_EMBS + 1  # 27 features
FP = 32            # feature pitch (27 + 5 pad)
NPAIR = NF * (NF - 1) // 2  # 351

_CACHE = {}


def build(bc=BC):
    import concourse.bacc as bacc
    import concourse.mybir as mybir
    from concourse.tile import TileContext
    from concourse.masks import make_identity

    fp16 = mybir.dt.float16
    fp32 = mybir.dt.float32

    nc = bacc.Bacc("TRN2", target_bir_lowering=False, debug=False)
    dense_t = nc.dram_tensor("dense", (bc, D), fp32, kind="ExternalInput")
    embs_t = nc.dram_tensor("embs", (bc, NUM_EMBS, D), fp32, kind="ExternalInput")
    out_t = nc.dram_tensor("out", (bc, NPAIR), fp32, kind="ExternalOutput")

    nblk = bc // BLK
    assert nblk % 2 == 0
    QBLK = min(4, nblk)  # blocks per quarter-group (Zs/scratch granularity)
    QG = 16              # 4-sample groups per PSUM Z tile

    with TileContext(nc) as tc:
        with (
            tc.tile_pool(name="consts", bufs=1) as consts,
            tc.tile_pool(name="xin", bufs=2) as xpool,
            tc.tile_pool(name="tt", bufs=3) as ttpool,
            tc.tile_pool(name="zsb", bufs=2) as zpool,
            tc.tile_pool(name="zb", bufs=3) as zbpool,
            tc.tile_pool(name="pk", bufs=3) as pkpool,
            tc.tile_pool(name="tp", bufs=4, space="PSUM") as tppool,
            tc.tile_pool(name="zp", bufs=3, space="PSUM") as zppool,
            tc.tile_pool(name="dscr", bufs=2, space="DRAM") as dpool,
        ):
            ident = consts.tile([128, 128], fp16)
            make_identity(nc, ident)

            dview = dense_t.ap()  # [bc, 128]
            eview = embs_t.ap().rearrange("b f d -> b (f d)")  # [bc, 3328]
            oview = out_t.ap()  # [bc, 351]

            X = None
            for qtr in range(nblk // QBLK):
                # Z results for the quarter, [(g,m) part, (blk, q, n)], fp16
                # (cast from fp32 PSUM during the ACT copy)
                Zs = zpool.tile([128, QBLK * (BLK // 4) * FP], fp16, tag="Zs")
                for pairi in range(max(1, QBLK // 2)):
                    npair = min(2, QBLK)
                    tts = []
                    # ---- phase 1: load + transpose for the block pair ----
                    for sub in range(npair):
                        blki = pairi * 2 + sub
                        blk = qtr * QBLK + blki
                        b0 = blk * BLK
                        if blk % 2 == 0:
                            X = xpool.tile([BLK, 2 * NF * D], fp16, tag="X")
                            dsrc = dview[b0 : b0 + 2 * BLK].rearrange(
                                "(t b) d -> b t d", t=2
                            )  # [128, 2, 128]
                            xd = X.rearrange("b (t c) -> b t c", t=2)
                            nc.gpsimd.dma_start(out=xd[:, :, 0:D], in_=dsrc)
                            esrc = eview[b0 : b0 + 2 * BLK].rearrange(
                                "(t b) c -> b t c", t=2
                            )  # [128, 2, 3328]
                            nc.gpsimd.dma_start(out=xd[:, :, D:], in_=esrc)
                        xoff = (blk % 2) * NF * D

                        Tt = ttpool.tile([128, FP * D], fp16, tag="Tt")
                        # zero pad features f=27..31 (cols 3456:4096)
                        nc.gpsimd.memset(Tt[:, NF * D :], 0.0)
                        nchunk = (NF + 7) // 8
                        for ci in range(nchunk):
                            c0 = ci * 8
                            cf = min(8, NF - c0)
                            tp = tppool.tile([128, 8 * BLK], fp16, tag="tp")
                            for j in range(cf):
                                f = c0 + j
                                nc.tensor.transpose(
                                    tp[:, j * BLK : (j + 1) * BLK],
                                    X[:, xoff + f * D : xoff + (f + 1) * D],
                                    ident,
                                )
                            dst = Tt[:, c0 * BLK : (c0 + cf) * BLK]
                            if ci % 4 < 3:
                                nc.vector.tensor_copy(
                                    out=dst, in_=tp[:, : cf * BLK]
                                )
                            else:
                                nc.scalar.copy(dst, tp[:, : cf * BLK])
                        tts.append((blki, Tt))

                    # ---- phase 2: per-sample Gram matmuls (dense PE burst) --
                    for blki, Tt in tts:
                        Ttr = Tt.rearrange("d (f b) -> d b f", b=BLK)
                        nq = BLK // 4  # 32 groups of 4 samples
                        for qt in range(0, nq, QG):
                            zp = zppool.tile([128, QG * FP], fp32, tag="zp")
                            for q in range(QG):
                                for g in range(4):
                                    bloc = (qt + q) * 4 + g
                                    op = Ttr[:, bloc, :]  # [128 d, 32 f]
                                    nc.tensor.matmul(
                                        zp[
                                            32 * g : 32 * (g + 1),
                                            q * FP : (q + 1) * FP,
                                        ],
                                        op,
                                        op,
                                        start=True,
                                        stop=True,
                                        tile_position=(0, 32 * g),
                                    )
                            # copy Z PSUM -> SBUF quarter buffer, cast fp16
                            zcol0 = (blki * nq + qt) * FP
                            zdst = Zs[:, zcol0 : zcol0 + QG * FP]
                            if qt == 0:
                                nc.scalar.copy(zdst, zp[:, : QG * FP])
                            else:
                                nc.vector.tensor_copy(
                                    out=zdst, in_=zp[:, : QG * FP]
                                )

                # ---- bounce Z to m-major DRAM scratch (16KB runs) ----
                # scratch layout [g, m, q, n]; one SWDGE DMA per g keeps the
                # partition base 32-aligned and runs (q,n)-contiguous.
                nqq = QBLK * (BLK // 4)  # 4-sample groups in the quarter
                Zr = Zs.rearrange("p (q n) -> p q n", n=FP)  # [128, nqq, 32]
                scr = dpool.tile([4, NF * nqq * FP], fp16, tag="scr")
                sc4 = scr.rearrange("g (m q n) -> g m q n", q=nqq, n=FP)
                for g in range(4):
                    nc.gpsimd.dma_start(
                        out=sc4[g], in_=Zr[32 * g : 32 * g + NF, :, :]
                    )

                # ---- reload as [(g, qlo) part, (t, m, n)] in one wide DMA ----
                # sample s = q*4+g with q = t*32+qlo -> partition g*32+qlo,
                # column block t. 128B runs, ~14k descriptors -> wide fanout.
                Zb = zbpool.tile([128, QBLK * NF * FP], fp16, tag="Zb")
                zb5 = Zb.rearrange(
                    "(g qlo) (t m n) -> g qlo t m n", g=4, t=QBLK, n=FP
                )  # [4, 32, t, 27, 32]
                sc5 = sc4.rearrange(
                    "g m (t qlo) n -> g qlo t m n", t=QBLK
                )  # [4, 32, t, 27, 32]
                for g in range(4):
                    for t in range(QBLK):
                        nc.sync.dma_start(out=zb5[g, :, t], in_=sc5[g, :, t])

                # ---- pack triu (QBLK tiles wide per copy, DVE) ----
                Pk = pkpool.tile([128, QBLK * NPAIR], fp32, tag="Pk")
                zbp = Zb.rearrange(
                    "p (t c) -> p t c", t=QBLK
                )  # [128, t, 864+pad]
                pkp = Pk.rearrange("p (t c) -> p t c", t=QBLK)  # [128, t, 351]
                off = 0
                for m in range(NF - 1):
                    ln = NF - 1 - m
                    src = zbp[:, :, m * FP + m + 1 : m * FP + NF]
                    dst = pkp[:, :, off : off + ln]
                    # fp16 -> fp32 cast happens in the copy
                    if m % 4 == 3:
                        nc.scalar.copy(dst, src)
                    else:
                        nc.vector.tensor_copy(out=dst, in_=src)
                    off += ln

                # ---- output: per-g HWDGE DMA, 1404B runs ----
                b0q = qtr * QBLK * BLK
                ovq = oview[b0q : b0q + QBLK * BLK].rearrange(
                    "(t qlo g) p -> g qlo t p", g=4, t=QBLK
                )  # [4, 32, t, 351]
                pk4 = pkp.rearrange("(g qlo) t c -> g qlo t c", g=4)
                for g in range(4):
                    eng = nc.sync if g % 2 == 0 else nc.scalar
                    eng.dma_start(out=ovq[g], in_=pk4[g])

    nc.compile()
    return nc


def _get(bc=BC):
    if bc not in _CACHE:
        _CACHE[bc] = build(bc)
    return _CACHE[bc]


def kernel(dense: np.ndarray, embs: np.ndarray) -> np.ndarray:
    from concourse import bass_utils

    dense = np.ascontiguousarray(np.asarray(dense, dtype=np.float32))
    embs = np.ascontiguousarray(np.asarray(embs, dtype=np.float32))
    assert dense.shape == (B, D) and embs.shape == (B, NUM_EMBS, D)

    nc = _get()
    dsh = dense.reshape(N_CORES, BC, D)
    esh = embs.reshape(N_CORES, BC, NUM_EMBS, D)
    in_maps = [{"dense": dsh[i], "embs": esh[i]} for i in range(N_CORES)]
    res = bass_utils.run_bass_kernel_spmd(nc, in_maps, core_ids=list(range(N_CORES)))
    return np.concatenate([r["out"] for r in res.results], axis=0)


# revision 36
# speedup vs baseline: 132.7674x; 115.7599x over previous
"""DLRM dot-interaction kernel for Trainium2 (8 NeuronCores, batch-sharded).

Per sample b: T = concat(dense[b], embs[b]) -> [27, 128]; Z = T @ T^T;
output = strict upper triangle of Z -> [351] fp32.

Per-core plan (2048 samples, 16 blocks of 128):
  - SWDGE cast-DMA loads 2 blocks at a time as [128 b, (f,d)] fp16.
  - PE transposes each [128 b, 128 d] feature slab into PSUM; DVE/ACT copy
    into f-major Tt [128 d, f*128+b] fp16 (contiguous copies).
  - Per-sample fp16 matmul: lhsT = rhs = strided AP [128 d, 32 f] (27 real
    features + 5 zero pads); out -> PSUM [32, 32] at partition 32*(b%4)
    (col-group tiling, 4 samples per PSUM partition dim).
  - ACT copies Z PSUM -> SBUF Zs [(g,m) part, (blk,q,n)] fp32, half-core span.
  - Triu extraction: SWDGE bounces Zs to DRAM scratch (full rows, big
    descriptors); then per (m, half) one HWDGE DRAM->DRAM gather DMA with
    1024 descriptors (spreads over ~8-16 DMA engines) packs z[m, m+1:27]
    runs into out[b, off_m:...].
"""

import numpy as np

B, NUM_EMBS, D = 16384, 26, 128
N_CORES = 8
BC = B // N_CORES  # 2048 samples per core
BLK = 128          # samples per block
NF = NUM_EMBS + 1  # 27 features
FP = 32            # feature pitch (27 + 5 pad)
NPAIR = NF * (NF - 1) // 2  # 351

_CACHE = {}


def build(bc=BC):
    import concourse.bacc as bacc
    import concourse.mybir as mybir
    from concourse.tile import TileContext
    from concourse.masks import make_identity

    fp16 = mybir.dt.float16
    fp32 = mybir.dt.float32

    nc = bacc.Bacc("TRN2", target_bir_lowering=False, debug=False)
    dense_t = nc.dram_tensor("dense", (bc, D), fp32, kind="ExternalInput")
    embs_t = nc.dram_tensor("embs", (bc, NUM_EMBS, D), fp32, kind="ExternalInput")
    out_t = nc.dram_tensor("out", (bc, NPAIR), fp32, kind="ExternalOutput")

    nblk = bc // BLK
    assert nblk % 2 == 0
    QBLK = min(4, nblk)  # blocks per quarter-group (Zs/scratch granularity)
    QG = 16              # 4-sample groups per PSUM Z tile

    with TileContext(nc) as tc:
        with (
            tc.tile_pool(name="consts", bufs=1) as consts,
            tc.tile_pool(name="xin", bufs=3) as xpool,
            tc.tile_pool(name="tt", bufs=3) as ttpool,
            tc.tile_pool(name="zsb", bufs=2) as zpool,
            tc.tile_pool(name="zb", bufs=3) as zbpool,
            tc.tile_pool(name="pk", bufs=3) as pkpool,
            tc.tile_pool(name="tp", bufs=4, space="PSUM") as tppool,
            tc.tile_pool(name="zp", bufs=4, space="PSUM") as zppool,
            tc.tile_pool(name="dscr", bufs=2, space="DRAM") as dpool,
        ):
            ident = consts.tile([128, 128], fp16)
            make_identity(nc, ident)

            dview = dense_t.ap()  # [bc, 128]
            eview = embs_t.ap().rearrange("b f d -> b (f d)")  # [bc, 3328]
            oview = out_t.ap()  # [bc, 351]

            X = None
            for qtr in range(nblk // QBLK):
                # Z results for the quarter, [(g,m) part, (blk, q, n)], fp16
                # (cast from fp32 PSUM during the ACT copy)
                Zs = zpool.tile([128, QBLK * (BLK // 4) * FP], fp16, tag="Zs")
                for pairi in range(max(1, QBLK // 2)):
                    npair = min(2, QBLK)
                    tts = []
                    # ---- phase 1: load + transpose for the block pair ----
                    for sub in range(npair):
                        blki = pairi * 2 + sub
                        blk = qtr * QBLK + blki
                        b0 = blk * BLK
                        if blk % 2 == 0:
                            X = xpool.tile([BLK, 2 * NF * D], fp16, tag="X")
                            dsrc = dview[b0 : b0 + 2 * BLK].rearrange(
                                "(t b) d -> b t d", t=2
                            )  # [128, 2, 128]
                            xd = X.rearrange("b (t c) -> b t c", t=2)
                            nc.gpsimd.dma_start(out=xd[:, :, 0:D], in_=dsrc)
                            esrc = eview[b0 : b0 + 2 * BLK].rearrange(
                                "(t b) c -> b t c", t=2
                            )  # [128, 2, 3328]
                            nc.gpsimd.dma_start(out=xd[:, :, D:], in_=esrc)
                        xoff = (blk % 2) * NF * D

                        Tt = ttpool.tile([128, FP * D], fp16, tag="Tt")
                        # zero pad features f=27..31 (cols 3456:4096)
                        nc.gpsimd.memset(Tt[:, NF * D :], 0.0)
                        nchunk = (NF + 7) // 8
                        for ci in range(nchunk):
                            c0 = ci * 8
                            cf = min(8, NF - c0)
                            tp = tppool.tile([128, 8 * BLK], fp16, tag="tp")
                            for j in range(cf):
                                f = c0 + j
                                nc.tensor.transpose(
                                    tp[:, j * BLK : (j + 1) * BLK],
                                    X[:, xoff + f * D : xoff + (f + 1) * D],
                                    ident,
                                )
                            dst = Tt[:, c0 * BLK : (c0 + cf) * BLK]
                            if ci % 4 < 3:
                                nc.vector.tensor_copy(
                                    out=dst, in_=tp[:, : cf * BLK]
                                )
                            else:
                                nc.scalar.copy(dst, tp[:, : cf * BLK])
                        tts.append((blki, Tt))

                    # ---- phase 2: per-sample Gram matmuls (dense PE burst) --
                    for blki, Tt in tts:
                        Ttr = Tt.rearrange("d (f b) -> d b f", b=BLK)
                        nq = BLK // 4  # 32 groups of 4 samples
                        for qt in range(0, nq, QG):
                            zp = zppool.tile([128, QG * FP], fp32, tag="zp")
                            for q in range(QG):
                                for g in range(4):
                                    bloc = (qt + q) * 4 + g
                                    op = Ttr[:, bloc, :]  # [128 d, 32 f]
                                    nc.tensor.matmul(
                                        zp[
                                            32 * g : 32 * (g + 1),
                                            q * FP : (q + 1) * FP,
                                        ],
                                        op,
                                        op,
                                        start=True,
                                        stop=True,
                                        tile_position=(0, 32 * g),
                                    )
                            # copy Z PSUM -> SBUF quarter buffer, cast fp16
                            zcol0 = (blki * nq + qt) * FP
                            zdst = Zs[:, zcol0 : zcol0 + QG * FP]
                            if qt == 0:
                                nc.scalar.copy(zdst, zp[:, : QG * FP])
                            else:
                                nc.vector.tensor_copy(
                                    out=zdst, in_=zp[:, : QG * FP]
                                )

                # ---- bounce Z to m-major DRAM scratch (16KB runs) ----
                # scratch layout [g, m, q, n]; one SWDGE DMA per g keeps the
                # partition base 32-aligned and runs (q,n)-contiguous.
                nqq = QBLK * (BLK // 4)  # 4-sample groups in the quarter
                Zr = Zs.rearrange("p (q n) -> p q n", n=FP)  # [128, nqq, 32]
                scr = dpool.tile([4, NF * nqq * FP], fp16, tag="scr")
                sc4 = scr.rearrange("g (m q n) -> g m q n", q=nqq, n=FP)
                for g in range(4):
                    nc.gpsimd.dma_start(
                        out=sc4[g], in_=Zr[32 * g : 32 * g + NF, :, :]
                    )

                # ---- reload as [(g, qlo) part, (t, m, n)] in one wide DMA ----
                # sample s = q*4+g with q = t*32+qlo -> partition g*32+qlo,
                # column block t. 128B runs, ~14k descriptors -> wide fanout.
                Zb = zbpool.tile([128, QBLK * NF * FP], fp16, tag="Zb")
                zb5 = Zb.rearrange(
                    "(g qlo) (t m n) -> g qlo t m n", g=4, t=QBLK, n=FP
                )  # [4, 32, t, 27, 32]
                sc5 = sc4.rearrange(
                    "g m (t qlo) n -> g qlo t m n", t=QBLK
                )  # [4, 32, t, 27, 32]
                for g in range(4):
                    for t in range(QBLK):
                        nc.sync.dma_start(out=zb5[g, :, t], in_=sc5[g, :, t])

                # ---- pack triu (QBLK tiles wide per copy, DVE) ----
                Pk = pkpool.tile([128, QBLK * NPAIR], fp32, tag="Pk")
                zbp = Zb.rearrange(
                    "p (t c) -> p t c", t=QBLK
                )  # [128, t, 864+pad]
                pkp = Pk.rearrange("p (t c) -> p t c", t=QBLK)  # [128, t, 351]
                off = 0
                for m in range(NF - 1):
                    ln = NF - 1 - m
                    src = zbp[:, :, m * FP + m + 1 : m * FP + NF]
                    dst = pkp[:, :, off : off + ln]
                    # fp16 -> fp32 cast happens in the copy
                    if m % 4 == 3:
                        nc.scalar.copy(dst, src)
                    else:
                        nc.vector.tensor_copy(out=dst, in_=src)
                    off += ln

                # ---- output: per-g HWDGE DMA, 1404B runs ----
                b0q = qtr * QBLK * BLK
                ovq = oview[b0q : b0q + QBLK * BLK].rearrange(
                    "(t qlo g) p -> g qlo t p", g=4, t=QBLK
                )  # [4, 32, t, 351]
                pk4 = pkp.rearrange("(g qlo) t c -> g qlo t c", g=4)
                for g in range(4):
                    eng = nc.sync if g % 2 == 0 else nc.scalar
                    eng.dma_start(out=ovq[g], in_=pk4[g])

    nc.compile()
    return nc


def _get(bc=BC):
    if bc not in _CACHE:
        _CACHE[bc] = build(bc)
    return _CACHE[bc]


def kernel(dense: np.ndarray, embs: np.ndarray) -> np.ndarray:
    from concourse import bass_utils

    dense = np.ascontiguousarray(np.asarray(dense, dtype=np.float32))
    embs = np.ascontiguousarray(np.asarray(embs, dtype=np.float32))
    assert dense.shape == (B, D) and embs.shape == (B, NUM_EMBS, D)

    nc = _get()
    dsh = dense.reshape(N_CORES, BC, D)
    esh = embs.reshape(N_CORES, BC, NUM_EMBS, D)
    in_maps = [{"dense": dsh[i], "embs": esh[i]} for i in range(N_CORES)]
    res = bass_utils.run_bass_kernel_spmd(nc, in_maps, core_ids=list(range(N_CORES)))
    return np.concatenate([r["out"] for r in res.results], axis=0)


# revision 38
# speedup vs baseline: 139.3236x; 1.0494x over previous
"""DLRM dot-interaction kernel for Trainium2 (8 NeuronCores, batch-sharded).

Per sample b: T = concat(dense[b], embs[b]) -> [27, 128]; Z = T @ T^T;
output = strict upper triangle of Z -> [351] fp32.

Per-core plan (2048 samples, 16 blocks of 128):
  - SWDGE cast-DMA loads 2 blocks at a time as [128 b, (f,d)] fp16.
  - PE transposes each [128 b, 128 d] feature slab into PSUM; DVE/ACT copy
    into f-major Tt [128 d, f*128+b] fp16 (contiguous copies).
  - Per-sample fp16 matmul: lhsT = rhs = strided AP [128 d, 32 f] (27 real
    features + 5 zero pads); out -> PSUM [32, 32] at partition 32*(b%4)
    (col-group tiling, 4 samples per PSUM partition dim).
  - ACT copies Z PSUM -> SBUF Zs [(g,m) part, (blk,q,n)] fp32, half-core span.
  - Triu extraction: SWDGE bounces Zs to DRAM scratch (full rows, big
    descriptors); then per (m, half) one HWDGE DRAM->DRAM gather DMA with
    1024 descriptors (spreads over ~8-16 DMA engines) packs z[m, m+1:27]
    runs into out[b, off_m:...].
"""

import numpy as np

B, NUM_EMBS, D = 16384, 26, 128
N_CORES = 8
BC = B // N_CORES  # 2048 samples per core
BLK = 128          # samples per block
NF = NUM_EMBS + 1  # 27 features
FP = 32            # feature pitch (27 + 5 pad)
NPAIR = NF * (NF - 1) // 2  # 351

_CACHE = {}


def build(bc=BC):
    import concourse.bacc as bacc
    import concourse.mybir as mybir
    from concourse.tile import TileContext
    from concourse.masks import make_identity

    fp16 = mybir.dt.float16
    fp32 = mybir.dt.float32

    nc = bacc.Bacc("TRN2", target_bir_lowering=False, debug=False)
    dense_t = nc.dram_tensor("dense", (bc, D), fp32, kind="ExternalInput")
    embs_t = nc.dram_tensor("embs", (bc, NUM_EMBS, D), fp32, kind="ExternalInput")
    out_t = nc.dram_tensor("out", (bc, NPAIR), fp32, kind="ExternalOutput")

    nblk = bc // BLK
    assert nblk % 2 == 0
    QBLK = min(4, nblk)  # blocks per quarter-group (Zs/scratch granularity)
    QG = 16              # 4-sample groups per PSUM Z tile

    with TileContext(nc) as tc:
        with (
            tc.tile_pool(name="consts", bufs=1) as consts,
            tc.tile_pool(name="xin", bufs=3) as xpool,
            tc.tile_pool(name="tt", bufs=4) as ttpool,
            tc.tile_pool(name="zsb", bufs=3) as zpool,
            tc.tile_pool(name="zb", bufs=3) as zbpool,
            tc.tile_pool(name="pk", bufs=3) as pkpool,
            tc.tile_pool(name="tp", bufs=4, space="PSUM") as tppool,
            tc.tile_pool(name="zp", bufs=4, space="PSUM") as zppool,
            tc.tile_pool(name="dscr", bufs=3, space="DRAM") as dpool,
        ):
            ident = consts.tile([128, 128], fp16)
            make_identity(nc, ident)

            dview = dense_t.ap()  # [bc, 128]
            eview = embs_t.ap().rearrange("b f d -> b (f d)")  # [bc, 3328]
            oview = out_t.ap()  # [bc, 351]

            X = None
            for qtr in range(nblk // QBLK):
                # Z results for the quarter, [(g,m) part, (blk, q, n)], fp16
                # (cast from fp32 PSUM during the ACT copy)
                Zs = zpool.tile([128, QBLK * (BLK // 4) * FP], fp16, tag="Zs")
                for pairi in range(max(1, QBLK // 2)):
                    npair = min(2, QBLK)
                    tts = []
                    # ---- phase 1: load + transpose for the block pair ----
                    for sub in range(npair):
                        blki = pairi * 2 + sub
                        blk = qtr * QBLK + blki
                        b0 = blk * BLK
                        if blk % 2 == 0:
                            X = xpool.tile([BLK, 2 * NF * D], fp16, tag="X")
                            dsrc = dview[b0 : b0 + 2 * BLK].rearrange(
                                "(t b) d -> b t d", t=2
                            )  # [128, 2, 128]
                            xd = X.rearrange("b (t c) -> b t c", t=2)
                            nc.gpsimd.dma_start(out=xd[:, :, 0:D], in_=dsrc)
                            esrc = eview[b0 : b0 + 2 * BLK].rearrange(
                                "(t b) c -> b t c", t=2
                            )  # [128, 2, 3328]
                            nc.gpsimd.dma_start(out=xd[:, :, D:], in_=esrc)
                        xoff = (blk % 2) * NF * D

                        Tt = ttpool.tile([128, FP * D], fp16, tag="Tt")
                        # zero pad features f=27..31 (cols 3456:4096)
                        nc.gpsimd.memset(Tt[:, NF * D :], 0.0)
                        nchunk = (NF + 7) // 8
                        for ci in range(nchunk):
                            c0 = ci * 8
                            cf = min(8, NF - c0)
                            tp = tppool.tile([128, 8 * BLK], fp16, tag="tp")
                            for j in range(cf):
                                f = c0 + j
                                nc.tensor.transpose(
                                    tp[:, j * BLK : (j + 1) * BLK],
                                    X[:, xoff + f * D : xoff + (f + 1) * D],
                                    ident,
                                )
                            dst = Tt[:, c0 * BLK : (c0 + cf) * BLK]
                            if ci % 4 < 3:
                                nc.vector.tensor_copy(
                                    out=dst, in_=tp[:, : cf * BLK]
                                )
                            else:
                                nc.scalar.copy(dst, tp[:, : cf * BLK])
                        tts.append((blki, Tt))

                    # ---- phase 2: per-sample Gram matmuls (dense PE burst) --
                    for blki, Tt in tts:
                        Ttr = Tt.rearrange("d (f b) -> d b f", b=BLK)
                        nq = BLK // 4  # 32 groups of 4 samples
                        for qt in range(0, nq, QG):
                            zp = zppool.tile([128, QG * FP], fp32, tag="zp")
                            for q in range(QG):
                                for g in range(4):
                                    bloc = (qt + q) * 4 + g
                                    op = Ttr[:, bloc, :]  # [128 d, 32 f]
                                    nc.tensor.matmul(
                                        zp[
                                            32 * g : 32 * (g + 1),
                                            q * FP : (q + 1) * FP,
                                        ],
                                        op,
                                        op,
                                        start=True,
                                        stop=True,
                                        tile_position=(0, 32 * g),
                                    )
                            # copy Z PSUM -> SBUF quarter buffer, cast fp16
                            zcol0 = (blki * nq + qt) * FP
                            zdst = Zs[:, zcol0 : zcol0 + QG * FP]
                            if qt == 0:
                                nc.scalar.copy(zdst, zp[:, : QG * FP])
                            else:
                                nc.vector.tensor_copy(
                                    out=zdst, in_=zp[:, : QG * FP]
                                )

                # ---- bounce Z to m-major DRAM scratch (16KB runs) ----
                # scratch layout [g, m, q, n]; one SWDGE DMA per g keeps the
                # partition base 32-aligned and runs (q,n)-contiguous.
                nqq = QBLK * (BLK // 4)  # 4-sample groups in the quarter
                Zr = Zs.rearrange("p (q n) -> p q n", n=FP)  # [128, nqq, 32]
                scr = dpool.tile([4, NF * nqq * FP], fp16, tag="scr")
                sc4 = scr.rearrange("g (m q n) -> g m q n", q=nqq, n=FP)
                for g in range(4):
                    nc.gpsimd.dma_start(
                        out=sc4[g], in_=Zr[32 * g : 32 * g + NF, :, :]
                    )

                # ---- reload as [(g, qlo) part, (t, m, n)] in one wide DMA ----
                # sample s = q*4+g with q = t*32+qlo -> partition g*32+qlo,
                # column block t. 128B runs, ~14k descriptors -> wide fanout.
                Zb = zbpool.tile([128, QBLK * NF * FP], fp16, tag="Zb")
                zb5 = Zb.rearrange(
                    "(g qlo) (t m n) -> g qlo t m n", g=4, t=QBLK, n=FP
                )  # [4, 32, t, 27, 32]
                sc5 = sc4.rearrange(
                    "g m (t qlo) n -> g qlo t m n", t=QBLK
                )  # [4, 32, t, 27, 32]
                for g in range(4):
                    for t in range(QBLK):
                        nc.sync.dma_start(out=zb5[g, :, t], in_=sc5[g, :, t])

                # ---- pack triu (QBLK tiles wide per copy, DVE) ----
                Pk = pkpool.tile([128, QBLK * NPAIR], fp32, tag="Pk")
                zbp = Zb.rearrange(
                    "p (t c) -> p t c", t=QBLK
                )  # [128, t, 864+pad]
                pkp = Pk.rearrange("p (t c) -> p t c", t=QBLK)  # [128, t, 351]
                off = 0
                for m in range(NF - 1):
                    ln = NF - 1 - m
                    src = zbp[:, :, m * FP + m + 1 : m * FP + NF]
                    dst = pkp[:, :, off : off + ln]
                    # fp16 -> fp32 cast happens in the copy
                    if m % 4 == 3:
                        nc.scalar.copy(dst, src)
                    else:
                        nc.vector.tensor_copy(out=dst, in_=src)
                    off += ln

                # ---- output: per-g HWDGE DMA, 1404B runs ----
                b0q = qtr * QBLK * BLK
                ovq = oview[b0q : b0q + QBLK * BLK].rearrange(
                    "(t qlo g) p -> g qlo t p", g=4, t=QBLK
                )  # [4, 32, t, 351]
                pk4 = pkp.rearrange("(g qlo) t c -> g qlo t c", g=4)
                for g in range(4):
                    eng = nc.sync if g % 2 == 0 else nc.scalar
                    eng.dma_start(out=ovq[g], in_=pk4[g])

    nc.compile()
    return nc


def _get(bc=BC):
    if bc not in _CACHE:
        _CACHE[bc] = build(bc)
    return _CACHE[bc]


def kernel(dense: np.ndarray, embs: np.ndarray) -> np.ndarray:
    from concourse import bass_utils

    dense = np.ascontiguousarray(np.asarray(dense, dtype=np.float32))
    embs = np.ascontiguousarray(np.asarray(embs, dtype=np.float32))
    assert dense.shape == (B, D) and embs.shape == (B, NUM_EMBS, D)

    nc = _get()
    dsh = dense.reshape(N_CORES, BC, D)
    esh = embs.reshape(N_CORES, BC, NUM_EMBS, D)
    in_maps = [{"dense": dsh[i], "embs": esh[i]} for i in range(N_CORES)]
    res = bass_utils.run_bass_kernel_spmd(nc, in_maps, core_ids=list(range(N_CORES)))
    return np.concatenate([r["out"] for r in res.results], axis=0)
